# revision 7
# baseline (speedup 1.0000x reference)
"""AttentionRoPE Trainium2 kernel: 8-way tensor parallel over heads.

Reference computation (B=2, N=2048, DIM=1024, H=16 heads, D=64):
    qkv = x @ W_qkv.T ; q,k rotary-embedded; per-head softmax(q k^T / 8) v;
    out = attn @ W_proj.T + b_proj

Sharding: head-parallel. Core c owns heads {2c, 2c+1}: it computes its
384x1024 qkv weight shard, RoPE, full attention over all 4096 tokens for
its 2 heads, and a partial projection (its 128 attn channels x W_proj
columns).  Host sums the 8 partials and adds the bias.

Device pipeline per core (all matmul-heavy ops in float32r, which runs at
bf16 speed on TRN2 for moving dims >= 256; AV matmul in bf16):
  1. qkv token-major: psum(tok 128, 384) = xT_tile.T @ W_shardT
  2. RoPE on DVE.  Host pre-permutes W_q/W_k rows so each head's rotary
     pairs are deinterleaved ([evens | odds]), making every DVE op
     stride-1; cos/sin fed pre-arranged per token tile.
  3. q,k transposed to head-dim-major via PE transpose (f32r).
  4. Scores^T = k_chunk @ q^T per (batch, head); exp fused on ScalarE
     (scale=1/8) straight from PSUM, bf16 out.
  5. AV with a ones-column appended to v: av_ext = P^T.T @ [v|1] gives
     softmax numerator and denominator in one accumulated matmul chain.
  6. normalize rows (reciprocal + per-partition scale), transpose av,
     partial proj = avT.T @ W_projT_shard, DMA out.
"""
import os
import sys

for _p in ("/opt/trn_rl_repo", "/root/.axon_site/_ro/trn_rl_repo"):
    if os.path.isdir(_p) and _p not in sys.path:
        sys.path.append(_p)

import numpy as np
import ml_dtypes

import concourse.bass as bass
import concourse.mybir as mybir
import concourse.tile as tile
from concourse import bacc
from concourse import bass_utils

f32 = mybir.dt.float32
f32r = mybir.dt.float32r
bf16 = mybir.dt.bfloat16
AF = mybir.ActivationFunctionType

# problem constants
B, N, DIM = 2, 2048, 1024
NHEAD, D = 16, 64
T = B * N                   # 4096 tokens
P = 128
TT = T // P                 # 32 token tiles
TTB = N // P                # 16 token tiles per batch
KC = DIM // P               # 8 input-feature chunks
NCORE = 8
HPC = NHEAD // NCORE        # 2 heads per core
QKVF = 3 * HPC * D          # 384 qkv features per core
SCALE = D ** (-0.5)         # 1/8


# ---------------------------------------------------------------------------
# walrus flag patch: the default concourse invocation produces NEFFs whose
# NRT-side loads (ACT tables / DVE ucode / gpsimd libraries) never complete
# on this terminal; the explicit queue-semaphore config below matches the
# stock neuronx-cc invocation and fixes ACTIVATE/reciprocal/gpsimd hangs.
# ---------------------------------------------------------------------------
def _patched_bir_verify_and_optimise(tmpdir, inp="bir.json", outp="file.neff",
                                     arch=None, *, dve_root=None):
    from concourse.bass_utils import (get_walrus_driver, get_walrus_args,
                                      get_bir_arch, run_command)
    cmd = [
        get_walrus_driver(),
        "--pass",
        "birverifier,runtime_memory_reservation,lower_act,lower_dve,"
        "lower_ap_offset,codegen,neff_packager",
        "-i", inp,
        "--neff-output-filename", outp,
        "--enable-birsim=true",
        "--mem-mode=physical",
        "--policy=0",
        "--limit-io-queue=true",
        "--num-semaphores-per-queue", "16",
        "--num-hardware-queues-per-compiler-queue", "16",
        "--max-sem-num", "192",
        "--enable-ldw-opt=false",
        "--assign-static-dmas-to-sp=false",
        "--dram-page-size=256",
        "--enable-neff-debug-info=true",
        "--jobs", "8",
        *get_walrus_args(get_bir_arch(tmpdir, inp) if arch is None else arch,
                         tmpdir, dve_root=dve_root),
    ]
    run_command(cmd, cwd=tmpdir)
    return os.path.join(tmpdir, outp)


bass_utils.bir_verify_and_optimise = _patched_bir_verify_and_optimise


# ---------------------------------------------------------------------------
# device kernel builder (same SPMD program for all 8 cores)
# ---------------------------------------------------------------------------
def build_kernel():
    nc = bacc.Bacc()
    xt = nc.declare_dram_parameter("xt", [TT, KC, P, P], bf16, isOutput=False)
    wq = nc.declare_dram_parameter("wq", [P, KC, QKVF], bf16, isOutput=False)
    wp = nc.declare_dram_parameter("wp", [P, DIM], bf16, isOutput=False)
    rc = nc.declare_dram_parameter("rc", [P, TT, 32], f32, isOutput=False)
    rs = nc.declare_dram_parameter("rs", [P, TT, 32], f32, isOutput=False)
    ident = nc.declare_dram_parameter("ident", [P, P], bf16, isOutput=False)
    out = nc.declare_dram_parameter("out", [T, DIM], f32, isOutput=True)

    with tile.TileContext(nc) as tc:
        with (
            tc.tile_pool(name="const", bufs=1) as cpool,
            tc.tile_pool(name="work", bufs=2) as wpool,
            tc.tile_pool(name="xin", bufs=3) as xpool,
            tc.tile_pool(name="tmp", bufs=2) as tpool,
            tc.tile_pool(name="small", bufs=3) as spool,
            tc.tile_pool(name="ps", bufs=2, space="PSUM") as ps,
            tc.tile_pool(name="psm", bufs=3, space="PSUM") as psm,
            tc.tile_pool(name="psa", bufs=1, space="PSUM") as psa,
        ):
            # ---- constants -------------------------------------------------
            wq_sb = cpool.tile([P, KC, QKVF], bf16)
            nc.sync.dma_start(wq_sb[:], wq[:])
            wp_sb = cpool.tile([P, DIM], bf16)
            nc.sync.dma_start(wp_sb[:], wp[:])
            rc_sb = cpool.tile([P, TT, 32], f32)
            nc.sync.dma_start(rc_sb[:], rc[:])
            rs_sb = cpool.tile([P, TT, 32], f32)
            nc.sync.dma_start(rs_sb[:], rs[:])
            id_sb = cpool.tile([P, P], bf16)
            nc.sync.dma_start(id_sb[:], ident[:])

            # v with ones columns: [tok_tile, head*65 + d], col 64/129 == 1
            v_sb = cpool.tile([P, TT, 2 * (D + 1)], bf16)
            nc.vector.memset(v_sb[:], 1.0)

            # head-dim-major rotated q/k for the whole 4096 tokens
            qT_sb = cpool.tile([P, T], bf16)
            kT_sb = cpool.tile([P, T], bf16)
            # token-major (unnormalized) attention outputs, one batch in
            # flight at a time (bufs=2 so batch b+1 overlaps b's tail)
            # av_sb[p, j, h*64+d]
            # avT: head-dim-major for proj
            for b in range(B):
                jlo = b * TTB

                # ---- qkv + stage -------------------------------------
                # token tiles m in this batch; groups of 8 for RoPE
                qkst_tiles = []
                for g in range(TTB // 8):
                    qkst_g = wpool.tile([P, 8, 2, 2, 2, 32], f32, tag="qkst",
                                        name=f"qkst_{b}_{g}")
                    qkst_tiles.append(qkst_g)
                for mi in range(TTB):
                    m = jlo + mi
                    xtile = xpool.tile([P, KC, P], bf16, tag="xtile")
                    nc.sync.dma_start(
                        xtile[:], xt[m].rearrange("k p c -> p k c"))
                    pq = psm.tile([P, QKVF], f32, tag="misc")
                    for k in range(KC):
                        nc.tensor.matmul(
                            pq[:], xtile[:, k, :], wq_sb[:, k, :],
                            start=(k == 0), stop=(k == KC - 1))
                    # stage q,k (cols 0:256) and v (cols 256:384)
                    g, jg = mi // 8, mi % 8
                    nc.vector.tensor_copy(
                        qkst_tiles[g][:, jg], pq[:, 0:2 * HPC * D])
                    # v: psum cols 256+h*64+d -> v_sb[:, m, h*65+d]
                    vdst = v_sb[:, m].rearrange("p (h e) -> p h e", h=2)[:, :, 0:64]
                    vsrc = pq[:, 2 * HPC * D:].rearrange(
                        "p (h d) -> p h d", h=2)
                    nc.vector.tensor_copy(vdst, vsrc)

                # ---- RoPE --------------------------------------------
                qkrot_tiles = []
                for g in range(TTB // 8):
                    qkst = qkst_tiles[g]
                    qkrot = wpool.tile([P, 8, 2, 2, 2, 32], bf16, tag="qkrot")
                    qkrot_tiles.append(qkrot)
                    j0 = jlo + g * 8
                    cb = rc_sb[:, j0:j0 + 8, None, :].to_broadcast((P, 8, 2, 32))
                    sb_ = rs_sb[:, j0:j0 + 8, None, :].to_broadcast((P, 8, 2, 32))
                    for qk in range(2):
                        x0 = qkst[:, :, qk, :, 0, :]
                        x1 = qkst[:, :, qk, :, 1, :]
                        t0 = tpool.tile([P, 8, 2, 32], f32, tag="t0")
                        t1 = tpool.tile([P, 8, 2, 32], f32, tag="t1")
                        nc.vector.tensor_mul(t0[:], x0, cb)
                        nc.vector.tensor_mul(t1[:], x1, sb_)
                        nc.vector.tensor_sub(qkrot[:, :, qk, :, 0, :], t0[:], t1[:])
                        t2 = tpool.tile([P, 8, 2, 32], f32, tag="t2")
                        t3 = tpool.tile([P, 8, 2, 32], f32, tag="t3")
                        nc.vector.tensor_mul(t2[:], x0, sb_)
                        nc.vector.tensor_mul(t3[:], x1, cb)
                        nc.vector.tensor_add(qkrot[:, :, qk, :, 1, :], t2[:], t3[:])

                # ---- transpose q,k to head-dim-major -----------------
                for mi in range(TTB):
                    m = jlo + mi
                    qkrot = qkrot_tiles[mi // 8]
                    for qk in range(2):
                        trp = psm.tile([P, P], bf16, tag="misc")
                        nc.tensor.transpose(
                            trp[:], qkrot[:, mi % 8, qk], id_sb[:])
                        dst = qT_sb if qk == 0 else kT_sb
                        nc.vector.tensor_copy(dst[:, m * P:(m + 1) * P], trp[:])

                # ---- attention per (head, q-block) -------------------
                av_sb = wpool.tile([P, TTB, HPC * D], bf16, tag="av")
                for h in range(HPC):
                    prt = slice(h * D, (h + 1) * D)
                    for qb in range(4):
                        qc = b * N + qb * 512
                        expst = wpool.tile([P, 16, 512], bf16, tag="expst")
                        for g in range(8):
                            stp = ps.tile([P, 1024], f32, tag="st")
                            for s in range(2):
                                kc = g * 2 + s
                                nc.tensor.matmul(
                                    stp[:, s * 512:(s + 1) * 512],
                                    kT_sb[prt, b * N + kc * P: b * N + (kc + 1) * P],
                                    qT_sb[prt, qc:qc + 512],
                                    start=True, stop=True)
                            nc.scalar.activation(
                                expst[:, g * 2:(g + 1) * 2, :], stp[:],
                                AF.Exp, scale=SCALE)
                        for qs in range(4):
                            avp = psa.tile([P, D + 1], f32, tag="avp")
                            for kc in range(16):
                                nc.tensor.matmul(
                                    avp[:],
                                    expst[:, kc, qs * P:(qs + 1) * P],
                                    v_sb[:, b * TTB + kc,
                                         h * (D + 1):(h + 1) * (D + 1)],
                                    start=(kc == 0), stop=(kc == 15))
                            rec = spool.tile([P, 1], f32, tag="rec")
                            nc.vector.reciprocal(rec[:], avp[:, D:D + 1])
                            jj = qb * 4 + qs
                            nc.vector.tensor_scalar_mul(
                                av_sb[:, jj, h * D:(h + 1) * D],
                                avp[:, 0:D], rec[:])

                # ---- transpose av, proj, out -------------------------
                avT = wpool.tile([P, N], bf16, tag="avT")
                for jj in range(TTB):
                    trp = psm.tile([P, P], bf16, tag="misc")
                    nc.tensor.transpose(trp[:], av_sb[:, jj], id_sb[:])
                    nc.vector.tensor_copy(avT[:, jj * P:(jj + 1) * P], trp[:])
                for jj in range(TTB):
                    for n in range(2):
                        pp = psm.tile([P, 512], f32, tag="misc")
                        nc.tensor.matmul(
                            pp[:], avT[:, jj * P:(jj + 1) * P],
                            wp_sb[:, n * 512:(n + 1) * 512],
                            start=True, stop=True)
                        ostage = spool.tile([P, 512], f32, tag="ostage")
                        nc.vector.tensor_copy(ostage[:], pp[:])
                        nc.sync.dma_start(
                            out[(jlo + jj) * P:(jlo + jj + 1) * P,
                                n * 512:(n + 1) * 512],
                            ostage[:])

    nc.finalize()
    return nc


_CACHED = {}


def _get_kernel():
    if "nc" not in _CACHED:
        _CACHED["nc"] = build_kernel()
    return _CACHED["nc"]


# ---------------------------------------------------------------------------
# host-side sharding / gather
# ---------------------------------------------------------------------------
def _deint(base, h):
    """qkv row indices for head h with rotary pairs deinterleaved."""
    ev = [base + D * h + 2 * i for i in range(32)]
    od = [base + D * h + 2 * i + 1 for i in range(32)]
    return ev + od


def make_in_maps(x, freqs_cos, freqs_sin, W_qkv, W_proj):
    x = np.asarray(x, dtype=np.float32)
    fc = np.asarray(freqs_cos, dtype=np.float32)
    fs = np.asarray(freqs_sin, dtype=np.float32)
    W_qkv = np.asarray(W_qkv, dtype=np.float32)
    W_proj = np.asarray(W_proj, dtype=np.float32)

    xf = x.reshape(T, DIM)
    # xt[m, k, p, c] = x[m*128+c, k*128+p]
    xt = np.ascontiguousarray(
        xf.reshape(TT, P, KC, P).transpose(0, 2, 3, 1)).astype(ml_dtypes.bfloat16)

    # rope tables: token tile j, partition p -> batch token (j % TTB)*128+p
    tokn = (np.arange(TT)[None, :] % TTB) * P + np.arange(P)[:, None]  # (P, TT)
    rc = np.ascontiguousarray(fc[tokn])          # (P, TT, 32)
    rs = np.ascontiguousarray(fs[tokn])
    ident = np.eye(P, dtype=np.float32).astype(ml_dtypes.bfloat16)

    in_maps = []
    for c in range(NCORE):
        h0, h1 = 2 * c, 2 * c + 1
        q_rows = _deint(0, h0) + _deint(0, h1)
        k_rows = _deint(DIM, h0) + _deint(DIM, h1)
        v_rows = ([2 * DIM + D * h0 + d for d in range(D)] +
                  [2 * DIM + D * h1 + d for d in range(D)])
        W_shard = W_qkv[q_rows + k_rows + v_rows, :]          # (384, 1024)
        # wq[p, k, f] = W_shard[f, k*128+p]
        wq = np.ascontiguousarray(
            W_shard.T.reshape(KC, P, QKVF).transpose(1, 0, 2)).astype(ml_dtypes.bfloat16)
        wp = np.ascontiguousarray(
            W_proj[:, P * c:P * (c + 1)].T).astype(ml_dtypes.bfloat16)
        in_maps.append(dict(xt=xt, wq=wq, wp=wp, rc=rc, rs=rs, ident=ident))
    return in_maps


def kernel(x, freqs_cos, freqs_sin, W_qkv, W_proj, b_proj, _trace=False):
    from concourse.bass_utils import run_bass_kernel_spmd

    nc = _get_kernel()
    in_maps = make_in_maps(x, freqs_cos, freqs_sin, W_qkv, W_proj)
    res = run_bass_kernel_spmd(nc, in_maps, list(range(NCORE)), trace=_trace)
    acc = np.zeros((T, DIM), dtype=np.float32)
    for c in range(NCORE):
        acc += res.results[c]["out"]
    acc += np.asarray(b_proj, dtype=np.float32)[None, :]
    outv = acc.reshape(B, N, DIM)
    if _trace:
        return outv, res
    return outv


# revision 8
# speedup vs baseline: 1.2536x; 1.2536x over previous
"""AttentionRoPE Trainium2 kernel: 8-way tensor parallel over heads.

Reference computation (B=2, N=2048, DIM=1024, H=16 heads, D=64):
    qkv = x @ W_qkv.T ; q,k rotary-embedded; per-head softmax(q k^T / 8) v;
    out = attn @ W_proj.T + b_proj

Sharding: head-parallel. Core c owns heads {2c, 2c+1}: it computes its
384x1024 qkv weight shard, RoPE, full attention over all 4096 tokens for
its 2 heads, and a partial projection (its 128 attn channels x W_proj
columns).  Host sums the 8 partials and adds the bias.

Device pipeline per core (all matmul-heavy ops in float32r, which runs at
bf16 speed on TRN2 for moving dims >= 256; AV matmul in bf16):
  1. qkv token-major: psum(tok 128, 384) = xT_tile.T @ W_shardT
  2. RoPE on DVE.  Host pre-permutes W_q/W_k rows so each head's rotary
     pairs are deinterleaved ([evens | odds]), making every DVE op
     stride-1; cos/sin fed pre-arranged per token tile.
  3. q,k transposed to head-dim-major via PE transpose (f32r).
  4. Scores^T = k_chunk @ q^T per (batch, head); exp fused on ScalarE
     (scale=1/8) straight from PSUM, bf16 out.
  5. AV with a ones-column appended to v: av_ext = P^T.T @ [v|1] gives
     softmax numerator and denominator in one accumulated matmul chain.
  6. normalize rows (reciprocal + per-partition scale), transpose av,
     partial proj = avT.T @ W_projT_shard, DMA out.
"""
import os
import sys

for _p in ("/opt/trn_rl_repo", "/root/.axon_site/_ro/trn_rl_repo"):
    if os.path.isdir(_p) and _p not in sys.path:
        sys.path.append(_p)

import numpy as np
import ml_dtypes

import concourse.bass as bass
import concourse.mybir as mybir
import concourse.tile as tile
from concourse import bacc
from concourse import bass_utils

f32 = mybir.dt.float32
f32r = mybir.dt.float32r
bf16 = mybir.dt.bfloat16
AF = mybir.ActivationFunctionType

# problem constants
B, N, DIM = 2, 2048, 1024
NHEAD, D = 16, 64
T = B * N                   # 4096 tokens
P = 128
TT = T // P                 # 32 token tiles
TTB = N // P                # 16 token tiles per batch
KC = DIM // P               # 8 input-feature chunks
NCORE = 8
HPC = NHEAD // NCORE        # 2 heads per core
QKVF = 3 * HPC * D          # 384 qkv features per core
SCALE = D ** (-0.5)         # 1/8


# ---------------------------------------------------------------------------
# walrus flag patch: the default concourse invocation produces NEFFs whose
# NRT-side loads (ACT tables / DVE ucode / gpsimd libraries) never complete
# on this terminal; the explicit queue-semaphore config below matches the
# stock neuronx-cc invocation and fixes ACTIVATE/reciprocal/gpsimd hangs.
# ---------------------------------------------------------------------------
def _patched_bir_verify_and_optimise(tmpdir, inp="bir.json", outp="file.neff",
                                     arch=None, *, dve_root=None):
    from concourse.bass_utils import (get_walrus_driver, get_walrus_args,
                                      get_bir_arch, run_command)
    cmd = [
        get_walrus_driver(),
        "--pass",
        "birverifier,runtime_memory_reservation,lower_act,lower_dve,"
        "lower_ap_offset,codegen,neff_packager",
        "-i", inp,
        "--neff-output-filename", outp,
        "--enable-birsim=true",
        "--mem-mode=physical",
        "--policy=0",
        "--limit-io-queue=true",
        "--num-semaphores-per-queue", "16",
        "--num-hardware-queues-per-compiler-queue", "16",
        "--max-sem-num", "192",
        "--enable-ldw-opt=false",
        "--assign-static-dmas-to-sp=false",
        "--dram-page-size=256",
        "--enable-neff-debug-info=true",
        "--jobs", "8",
        *get_walrus_args(get_bir_arch(tmpdir, inp) if arch is None else arch,
                         tmpdir, dve_root=dve_root),
    ]
    run_command(cmd, cwd=tmpdir)
    return os.path.join(tmpdir, outp)


bass_utils.bir_verify_and_optimise = _patched_bir_verify_and_optimise


# ---------------------------------------------------------------------------
# device kernel builder (same SPMD program for all 8 cores)
# ---------------------------------------------------------------------------
def build_kernel():
    nc = bacc.Bacc()
    xt = nc.declare_dram_parameter("xt", [TT, KC, P, P], bf16, isOutput=False)
    wq = nc.declare_dram_parameter("wq", [P, KC, QKVF], bf16, isOutput=False)
    wp = nc.declare_dram_parameter("wp", [P, DIM], bf16, isOutput=False)
    rc = nc.declare_dram_parameter("rc", [P, TT, 32], f32, isOutput=False)
    rs = nc.declare_dram_parameter("rs", [P, TT, 32], f32, isOutput=False)
    ident = nc.declare_dram_parameter("ident", [P, P], bf16, isOutput=False)
    out = nc.declare_dram_parameter("out", [T, DIM], f32, isOutput=True)

    with tile.TileContext(nc) as tc:
        with (
            tc.tile_pool(name="const", bufs=1) as cpool,
            tc.tile_pool(name="work", bufs=2) as wpool,
            tc.tile_pool(name="xin", bufs=3) as xpool,
            tc.tile_pool(name="tmp", bufs=2) as tpool,
            tc.tile_pool(name="small", bufs=3) as spool,
            tc.tile_pool(name="ps", bufs=2, space="PSUM") as ps,
            tc.tile_pool(name="psm", bufs=3, space="PSUM") as psm,
            tc.tile_pool(name="psa", bufs=1, space="PSUM") as psa,
        ):
            # ---- constants -------------------------------------------------
            wq_sb = cpool.tile([P, KC, QKVF], bf16)
            nc.sync.dma_start(wq_sb[:], wq[:])
            wp_sb = cpool.tile([P, DIM], bf16)
            nc.sync.dma_start(wp_sb[:], wp[:])
            rc_sb = cpool.tile([P, TT, 32], f32)
            nc.sync.dma_start(rc_sb[:], rc[:])
            rs_sb = cpool.tile([P, TT, 32], f32)
            nc.sync.dma_start(rs_sb[:], rs[:])
            id_sb = cpool.tile([P, P], bf16)
            nc.sync.dma_start(id_sb[:], ident[:])

            # v with ones columns: [tok_tile, head*65 + d], col 64/129 == 1
            v_sb = cpool.tile([P, TT, 2 * (D + 1)], bf16)
            nc.vector.memset(v_sb[:], 1.0)

            # head-dim-major rotated q/k for the whole 4096 tokens
            qT_sb = cpool.tile([P, T], bf16)
            kT_sb = cpool.tile([P, T], bf16)

            qkst_tiles = {}
            qkrot_tiles = {}
            av_tiles = {}
            avT_tiles = {}

            # ---- emission helpers (software pipeline) ----------------
            def emit_qkv_m(b, mi):
                m = b * TTB + mi
                xtile = xpool.tile([P, KC, P], bf16, tag="xtile",
                                   name=f"xtile_{m}")
                nc.sync.dma_start(
                    xtile[:], xt[m].rearrange("k p c -> p k c"))
                pq = psm.tile([P, QKVF], f32, tag="misc", name=f"pq_{m}")
                for k in range(KC):
                    nc.tensor.matmul(
                        pq[:], xtile[:, k, :], wq_sb[:, k, :],
                        start=(k == 0), stop=(k == KC - 1))
                g, jg = mi // 8, mi % 8
                if (b, g) not in qkst_tiles:
                    qkst_tiles[(b, g)] = wpool.tile(
                        [P, 8, 2, 2, 2, 32], f32, tag="qkst",
                        name=f"qkst_{b}_{g}")
                nc.vector.tensor_copy(
                    qkst_tiles[(b, g)][:, jg], pq[:, 0:2 * HPC * D])
                vdst = v_sb[:, m].rearrange("p (h e) -> p h e", h=2)[:, :, 0:64]
                vsrc = pq[:, 2 * HPC * D:].rearrange("p (h d) -> p h d", h=2)
                nc.vector.tensor_copy(vdst, vsrc)

            def emit_rope_group(b, g):
                qkst = qkst_tiles[(b, g)]
                qkrot = wpool.tile([P, 8, 2, 2, 2, 32], bf16, tag="qkrot",
                                   name=f"qkrot_{b}_{g}")
                qkrot_tiles[(b, g)] = qkrot
                j0 = b * TTB + g * 8
                cb = rc_sb[:, j0:j0 + 8, None, :].to_broadcast((P, 8, 2, 32))
                sb_ = rs_sb[:, j0:j0 + 8, None, :].to_broadcast((P, 8, 2, 32))
                for qk in range(2):
                    x0 = qkst[:, :, qk, :, 0, :]
                    x1 = qkst[:, :, qk, :, 1, :]
                    t0 = tpool.tile([P, 8, 2, 32], f32, tag="t0")
                    t1 = tpool.tile([P, 8, 2, 32], f32, tag="t1")
                    nc.vector.tensor_mul(t0[:], x0, cb)
                    nc.vector.tensor_mul(t1[:], x1, sb_)
                    nc.vector.tensor_sub(qkrot[:, :, qk, :, 0, :], t0[:], t1[:])
                    t2 = tpool.tile([P, 8, 2, 32], f32, tag="t2")
                    t3 = tpool.tile([P, 8, 2, 32], f32, tag="t3")
                    nc.vector.tensor_mul(t2[:], x0, sb_)
                    nc.vector.tensor_mul(t3[:], x1, cb)
                    nc.vector.tensor_add(qkrot[:, :, qk, :, 1, :], t2[:], t3[:])

            def emit_tr_m(b, mi):
                m = b * TTB + mi
                qkrot = qkrot_tiles[(b, mi // 8)]
                for qk in range(2):
                    trp = psm.tile([P, P], bf16, tag="misc",
                                   name=f"trqk_{m}_{qk}")
                    nc.tensor.transpose(trp[:], qkrot[:, mi % 8, qk], id_sb[:])
                    dst = qT_sb if qk == 0 else kT_sb
                    nc.vector.tensor_copy(dst[:, m * P:(m + 1) * P], trp[:])

            def emit_st_block(b, h, qb):
                prt = slice(h * D, (h + 1) * D)
                qc = b * N + qb * 512
                expst = wpool.tile([P, 16, 512], bf16, tag="expst",
                                   name=f"expst_{b}_{h}_{qb}")
                for g in range(8):
                    stp = ps.tile([P, 1024], f32, tag="st")
                    for s_ in range(2):
                        kc = g * 2 + s_
                        nc.tensor.matmul(
                            stp[:, s_ * 512:(s_ + 1) * 512],
                            kT_sb[prt, b * N + kc * P: b * N + (kc + 1) * P],
                            qT_sb[prt, qc:qc + 512],
                            start=True, stop=True)
                    nc.scalar.activation(
                        expst[:, g * 2:(g + 1) * 2, :], stp[:],
                        AF.Exp, scale=SCALE)
                return expst

            def emit_av_block(b, h, qb, expst):
                if b not in av_tiles:
                    av_tiles[b] = wpool.tile([P, TTB, HPC * D], bf16,
                                             tag="av", name=f"av_{b}")
                av_sb = av_tiles[b]
                for qs in range(4):
                    avp = psa.tile([P, D + 1], f32, tag="avp")
                    for kc in range(16):
                        nc.tensor.matmul(
                            avp[:],
                            expst[:, kc, qs * P:(qs + 1) * P],
                            v_sb[:, b * TTB + kc,
                                 h * (D + 1):(h + 1) * (D + 1)],
                            start=(kc == 0), stop=(kc == 15))
                    rec = spool.tile([P, 1], f32, tag="rec")
                    nc.vector.reciprocal(rec[:], avp[:, D:D + 1])
                    jj = qb * 4 + qs
                    nc.vector.tensor_scalar_mul(
                        av_sb[:, jj, h * D:(h + 1) * D], avp[:, 0:D], rec[:])

            def emit_avtr(b, jj):
                if b not in avT_tiles:
                    avT_tiles[b] = wpool.tile([P, N], bf16, tag="avT",
                                              name=f"avT_{b}")
                trp = psm.tile([P, P], bf16, tag="misc", name=f"travt_{b}_{jj}")
                nc.tensor.transpose(trp[:], av_tiles[b][:, jj], id_sb[:])
                nc.vector.tensor_copy(
                    avT_tiles[b][:, jj * P:(jj + 1) * P], trp[:])

            def emit_proj(b, jj, n):
                pp = psm.tile([P, 512], f32, tag="misc", name=f"pp_{b}_{jj}_{n}")
                nc.tensor.matmul(
                    pp[:], avT_tiles[b][:, jj * P:(jj + 1) * P],
                    wp_sb[:, n * 512:(n + 1) * 512],
                    start=True, stop=True)
                ostage = spool.tile([P, 512], f32, tag="ostage")
                nc.vector.tensor_copy(ostage[:], pp[:])
                jl = b * TTB + jj
                nc.sync.dma_start(
                    out[jl * P:(jl + 1) * P, n * 512:(n + 1) * 512],
                    ostage[:])

            def front_work(b):
                for mi in range(TTB):
                    yield lambda mi=mi: emit_qkv_m(b, mi)
                    if mi % 8 == 7:
                        g = mi // 8
                        yield lambda g=g: emit_rope_group(b, g)
                        for mj in range(g * 8, g * 8 + 8):
                            yield lambda mj=mj: emit_tr_m(b, mj)

            def tail_work(b):
                for jj in range(TTB):
                    yield lambda jj=jj: emit_avtr(b, jj)
                    yield lambda jj=jj: emit_proj(b, jj, 0)
                    yield lambda jj=jj: emit_proj(b, jj, 1)

            def pump(it, nmax):
                done = 0
                for th in it:
                    th()
                    done += 1
                    if done >= nmax:
                        return

            # ---- schedule --------------------------------------------
            for th in front_work(0):
                th()
            blocks = [(b, h, qb) for b in range(B) for h in range(HPC)
                      for qb in range(4)]
            bg0 = front_work(1)
            bg1 = tail_work(0)
            prev = None
            for i, (b, h, qb) in enumerate(blocks):
                expst = emit_st_block(b, h, qb)
                if prev is not None:
                    emit_av_block(*prev)
                if b == 0:
                    pump(bg0, 5)
                else:
                    pump(bg0, 99)   # finish any leftover front(1)
                    pump(bg1, 6)
                prev = (b, h, qb, expst)
            emit_av_block(*prev)
            for th in bg1:
                th()
            for th in tail_work(1):
                th()

    nc.finalize()
    return nc


_CACHED = {}


def _get_kernel():
    if "nc" not in _CACHED:
        _CACHED["nc"] = build_kernel()
    return _CACHED["nc"]


# ---------------------------------------------------------------------------
# host-side sharding / gather
# ---------------------------------------------------------------------------
def _deint(base, h):
    """qkv row indices for head h with rotary pairs deinterleaved."""
    ev = [base + D * h + 2 * i for i in range(32)]
    od = [base + D * h + 2 * i + 1 for i in range(32)]
    return ev + od


def make_in_maps(x, freqs_cos, freqs_sin, W_qkv, W_proj):
    x = np.asarray(x, dtype=np.float32)
    fc = np.asarray(freqs_cos, dtype=np.float32)
    fs = np.asarray(freqs_sin, dtype=np.float32)
    W_qkv = np.asarray(W_qkv, dtype=np.float32)
    W_proj = np.asarray(W_proj, dtype=np.float32)

    xf = x.reshape(T, DIM)
    # xt[m, k, p, c] = x[m*128+c, k*128+p]
    xt = np.ascontiguousarray(
        xf.reshape(TT, P, KC, P).transpose(0, 2, 3, 1)).astype(ml_dtypes.bfloat16)

    # rope tables: token tile j, partition p -> batch token (j % TTB)*128+p
    tokn = (np.arange(TT)[None, :] % TTB) * P + np.arange(P)[:, None]  # (P, TT)
    rc = np.ascontiguousarray(fc[tokn])          # (P, TT, 32)
    rs = np.ascontiguousarray(fs[tokn])
    ident = np.eye(P, dtype=np.float32).astype(ml_dtypes.bfloat16)

    in_maps = []
    for c in range(NCORE):
        h0, h1 = 2 * c, 2 * c + 1
        q_rows = _deint(0, h0) + _deint(0, h1)
        k_rows = _deint(DIM, h0) + _deint(DIM, h1)
        v_rows = ([2 * DIM + D * h0 + d for d in range(D)] +
                  [2 * DIM + D * h1 + d for d in range(D)])
        W_shard = W_qkv[q_rows + k_rows + v_rows, :]          # (384, 1024)
        # wq[p, k, f] = W_shard[f, k*128+p]
        wq = np.ascontiguousarray(
            W_shard.T.reshape(KC, P, QKVF).transpose(1, 0, 2)).astype(ml_dtypes.bfloat16)
        wp = np.ascontiguousarray(
            W_proj[:, P * c:P * (c + 1)].T).astype(ml_dtypes.bfloat16)
        in_maps.append(dict(xt=xt, wq=wq, wp=wp, rc=rc, rs=rs, ident=ident))
    return in_maps


def kernel(x, freqs_cos, freqs_sin, W_qkv, W_proj, b_proj, _trace=False):
    from concourse.bass_utils import run_bass_kernel_spmd

    nc = _get_kernel()
    in_maps = make_in_maps(x, freqs_cos, freqs_sin, W_qkv, W_proj)
    res = run_bass_kernel_spmd(nc, in_maps, list(range(NCORE)), trace=_trace)
    acc = np.zeros((T, DIM), dtype=np.float32)
    for c in range(NCORE):
        acc += res.results[c]["out"]
    acc += np.asarray(b_proj, dtype=np.float32)[None, :]
    outv = acc.reshape(B, N, DIM)
    if _trace:
        return outv, res
    return outv


# revision 10
# speedup vs baseline: 1.5357x; 1.2250x over previous
"""AttentionRoPE Trainium2 kernel: 8-way tensor parallel over heads.

Reference computation (B=2, N=2048, DIM=1024, H=16 heads, D=64):
    qkv = x @ W_qkv.T ; q,k rotary-embedded; per-head softmax(q k^T / 8) v;
    out = attn @ W_proj.T + b_proj

Sharding: head-parallel. Core c owns heads {2c, 2c+1}: it computes its
384x1024 qkv weight shard, RoPE, full attention over all 4096 tokens for
its 2 heads, and a partial projection (its 128 attn channels x W_proj
columns).  Host sums the 8 partials and adds the bias.

Device pipeline per core (all matmul-heavy ops in float32r, which runs at
bf16 speed on TRN2 for moving dims >= 256; AV matmul in bf16):
  1. qkv token-major: psum(tok 128, 384) = xT_tile.T @ W_shardT
  2. RoPE on DVE.  Host pre-permutes W_q/W_k rows so each head's rotary
     pairs are deinterleaved ([evens | odds]), making every DVE op
     stride-1; cos/sin fed pre-arranged per token tile.
  3. q,k transposed to head-dim-major via PE transpose (f32r).
  4. Scores^T = k_chunk @ q^T per (batch, head); exp fused on ScalarE
     (scale=1/8) straight from PSUM, bf16 out.
  5. AV with a ones-column appended to v: av_ext = P^T.T @ [v|1] gives
     softmax numerator and denominator in one accumulated matmul chain.
  6. normalize rows (reciprocal + per-partition scale), transpose av,
     partial proj = avT.T @ W_projT_shard, DMA out.
"""
import os
import sys

for _p in ("/opt/trn_rl_repo", "/root/.axon_site/_ro/trn_rl_repo"):
    if os.path.isdir(_p) and _p not in sys.path:
        sys.path.append(_p)

import numpy as np
import ml_dtypes

import concourse.bass as bass
import concourse.mybir as mybir
import concourse.tile as tile
from concourse import bacc
from concourse import bass_utils

f32 = mybir.dt.float32
f32r = mybir.dt.float32r
bf16 = mybir.dt.bfloat16
AF = mybir.ActivationFunctionType

# problem constants
B, N, DIM = 2, 2048, 1024
NHEAD, D = 16, 64
T = B * N                   # 4096 tokens
P = 128
TT = T // P                 # 32 token tiles
TTB = N // P                # 16 token tiles per batch
KC = DIM // P               # 8 input-feature chunks
NCORE = 8
HPC = NHEAD // NCORE        # 2 heads per core
QKVF = 3 * HPC * D          # 384 qkv features per core
SCALE = D ** (-0.5)         # 1/8


# ---------------------------------------------------------------------------
# walrus flag patch: the default concourse invocation produces NEFFs whose
# NRT-side loads (ACT tables / DVE ucode / gpsimd libraries) never complete
# on this terminal; the explicit queue-semaphore config below matches the
# stock neuronx-cc invocation and fixes ACTIVATE/reciprocal/gpsimd hangs.
# ---------------------------------------------------------------------------
def _patched_bir_verify_and_optimise(tmpdir, inp="bir.json", outp="file.neff",
                                     arch=None, *, dve_root=None):
    from concourse.bass_utils import (get_walrus_driver, get_walrus_args,
                                      get_bir_arch, run_command)
    cmd = [
        get_walrus_driver(),
        "--pass",
        "birverifier,runtime_memory_reservation,lower_act,lower_dve,"
        "lower_ap_offset,codegen,neff_packager",
        "-i", inp,
        "--neff-output-filename", outp,
        "--enable-birsim=true",
        "--mem-mode=physical",
        "--policy=0",
        "--limit-io-queue=true",
        "--num-semaphores-per-queue", "16",
        "--num-hardware-queues-per-compiler-queue", "16",
        "--max-sem-num", "192",
        "--enable-ldw-opt=false",
        "--assign-static-dmas-to-sp=false",
        "--dram-page-size=256",
        "--enable-neff-debug-info=true",
        "--jobs", "8",
        *get_walrus_args(get_bir_arch(tmpdir, inp) if arch is None else arch,
                         tmpdir, dve_root=dve_root),
    ]
    run_command(cmd, cwd=tmpdir)
    return os.path.join(tmpdir, outp)


bass_utils.bir_verify_and_optimise = _patched_bir_verify_and_optimise


# ---------------------------------------------------------------------------
# device kernel builder (same SPMD program for all 8 cores)
# ---------------------------------------------------------------------------
def build_kernel():
    nc = bacc.Bacc()
    xt = nc.declare_dram_parameter("xt", [TT, KC, P, P], bf16, isOutput=False)
    wq = nc.declare_dram_parameter("wq", [P, KC, QKVF], bf16, isOutput=False)
    wp = nc.declare_dram_parameter("wp", [P, DIM], bf16, isOutput=False)
    rc = nc.declare_dram_parameter("rc", [P, TT, 32], f32, isOutput=False)
    rs = nc.declare_dram_parameter("rs", [P, TT, 32], f32, isOutput=False)
    ident = nc.declare_dram_parameter("ident", [P, P], bf16, isOutput=False)
    out = nc.declare_dram_parameter("out", [T, DIM], f32, isOutput=True)

    with tile.TileContext(nc) as tc:
        with (
            tc.tile_pool(name="const", bufs=1) as cpool,
            tc.tile_pool(name="work", bufs=2) as wpool,
            tc.tile_pool(name="xin", bufs=3) as xpool,
            tc.tile_pool(name="tmp", bufs=2) as tpool,
            tc.tile_pool(name="small", bufs=3) as spool,
            tc.tile_pool(name="ps", bufs=2, space="PSUM") as ps,
            tc.tile_pool(name="psm", bufs=3, space="PSUM") as psm,
            tc.tile_pool(name="psa", bufs=1, space="PSUM") as psa,
        ):
            # ---- constants -------------------------------------------------
            wq_sb = cpool.tile([P, KC, QKVF], bf16)
            nc.sync.dma_start(wq_sb[:], wq[:])
            wp_sb = cpool.tile([P, DIM], bf16)
            nc.sync.dma_start(wp_sb[:], wp[:])
            rc_sb = cpool.tile([P, TT, 32], f32)
            nc.sync.dma_start(rc_sb[:], rc[:])
            rs_sb = cpool.tile([P, TT, 32], f32)
            nc.sync.dma_start(rs_sb[:], rs[:])
            id_sb = cpool.tile([P, P], bf16)
            nc.sync.dma_start(id_sb[:], ident[:])

            # v with ones columns: [tok_tile, head*65 + d], col 64/129 == 1
            v_sb = cpool.tile([P, TT, 2 * (D + 1)], bf16)
            nc.vector.memset(v_sb[:], 1.0)

            # head-dim-major rotated q/k for the whole 4096 tokens
            qT_sb = cpool.tile([P, T], bf16)
            kT_sb = cpool.tile([P, T], bf16)

            qkst_tiles = {}
            qkrot_tiles = {}
            av_tiles = {}
            avT_tiles = {}

            # ---- emission helpers (software pipeline) ----------------
            def emit_qkv_m(b, mi):
                m = b * TTB + mi
                xtile = xpool.tile([P, KC, P], bf16, tag="xtile",
                                   name=f"xtile_{m}")
                nc.sync.dma_start(
                    xtile[:], xt[m].rearrange("k p c -> p k c"))
                pq = psm.tile([P, QKVF], f32, tag="misc", name=f"pq_{m}")
                for k in range(KC):
                    nc.tensor.matmul(
                        pq[:], xtile[:, k, :], wq_sb[:, k, :],
                        start=(k == 0), stop=(k == KC - 1))
                g, jg = mi // 8, mi % 8
                if (b, g) not in qkst_tiles:
                    qkst_tiles[(b, g)] = wpool.tile(
                        [P, 8, 2, 2, 2, 32], f32, tag="qkst",
                        name=f"qkst_{b}_{g}")
                nc.vector.tensor_copy(
                    qkst_tiles[(b, g)][:, jg], pq[:, 0:2 * HPC * D])
                vdst = v_sb[:, m].rearrange("p (h e) -> p h e", h=2)[:, :, 0:64]
                vsrc = pq[:, 2 * HPC * D:].rearrange("p (h d) -> p h d", h=2)
                nc.vector.tensor_copy(vdst, vsrc)

            def emit_rope_group(b, g):
                qkst = qkst_tiles[(b, g)]
                qkrot = wpool.tile([P, 8, 2, 2, 2, 32], bf16, tag="qkrot",
                                   name=f"qkrot_{b}_{g}")
                qkrot_tiles[(b, g)] = qkrot
                j0 = b * TTB + g * 8
                cb = rc_sb[:, j0:j0 + 8, None, :].to_broadcast((P, 8, 2, 32))
                sb_ = rs_sb[:, j0:j0 + 8, None, :].to_broadcast((P, 8, 2, 32))
                for qk in range(2):
                    x0 = qkst[:, :, qk, :, 0, :]
                    x1 = qkst[:, :, qk, :, 1, :]
                    t0 = tpool.tile([P, 8, 2, 32], f32, tag="t0")
                    t1 = tpool.tile([P, 8, 2, 32], f32, tag="t1")
                    nc.vector.tensor_mul(t0[:], x0, cb)
                    nc.vector.tensor_mul(t1[:], x1, sb_)
                    nc.vector.tensor_sub(qkrot[:, :, qk, :, 0, :], t0[:], t1[:])
                    t2 = tpool.tile([P, 8, 2, 32], f32, tag="t2")
                    t3 = tpool.tile([P, 8, 2, 32], f32, tag="t3")
                    nc.vector.tensor_mul(t2[:], x0, sb_)
                    nc.vector.tensor_mul(t3[:], x1, cb)
                    nc.vector.tensor_add(qkrot[:, :, qk, :, 1, :], t2[:], t3[:])

            def emit_tr_m(b, mi):
                m = b * TTB + mi
                qkrot = qkrot_tiles[(b, mi // 8)]
                for qk in range(2):
                    trp = psm.tile([P, P], bf16, tag="misc",
                                   name=f"trqk_{m}_{qk}")
                    nc.tensor.transpose(trp[:], qkrot[:, mi % 8, qk], id_sb[:])
                    dst = qT_sb if qk == 0 else kT_sb
                    nc.vector.tensor_copy(dst[:, m * P:(m + 1) * P], trp[:])

            def emit_st_block(b, h, qb):
                prt = slice(h * D, (h + 1) * D)
                qc = b * N + qb * 512
                expst = wpool.tile([P, 16, 512], bf16, tag="expst",
                                   name=f"expst_{b}_{h}_{qb}")
                for g in range(8):
                    stp = ps.tile([P, 1024], f32, tag="st")
                    for s_ in range(2):
                        kc = g * 2 + s_
                        nc.tensor.matmul(
                            stp[:, s_ * 512:(s_ + 1) * 512],
                            kT_sb[prt, b * N + kc * P: b * N + (kc + 1) * P],
                            qT_sb[prt, qc:qc + 512],
                            start=True, stop=True)
                    nc.scalar.activation(
                        expst[:, g * 2:(g + 1) * 2, :], stp[:],
                        AF.Exp, scale=SCALE)
                return expst

            def emit_av_chain(b, h, qb, expst, qs):
                if b not in av_tiles:
                    av_tiles[b] = wpool.tile([P, TTB, HPC * D], bf16,
                                             tag="av", name=f"av_{b}")
                av_sb = av_tiles[b]
                avp = psa.tile([P, D + 1], f32, tag="avp")
                for kc in range(16):
                    nc.tensor.matmul(
                        avp[:],
                        expst[:, kc, qs * P:(qs + 1) * P],
                        v_sb[:, b * TTB + kc,
                             h * (D + 1):(h + 1) * (D + 1)],
                        start=(kc == 0), stop=(kc == 15))
                rec = spool.tile([P, 1], f32, tag="rec")
                nc.vector.reciprocal(rec[:], avp[:, D:D + 1])
                jj = qb * 4 + qs
                nc.vector.tensor_scalar_mul(
                    av_sb[:, jj, h * D:(h + 1) * D], avp[:, 0:D], rec[:])

            def emit_avtr(b, jj):
                if b not in avT_tiles:
                    avT_tiles[b] = wpool.tile([P, N], bf16, tag="avT",
                                              name=f"avT_{b}")
                trp = psm.tile([P, P], bf16, tag="misc", name=f"travt_{b}_{jj}")
                nc.tensor.transpose(trp[:], av_tiles[b][:, jj], id_sb[:])
                nc.vector.tensor_copy(
                    avT_tiles[b][:, jj * P:(jj + 1) * P], trp[:])

            def emit_proj(b, jj, n):
                pp = psm.tile([P, 512], f32, tag="misc", name=f"pp_{b}_{jj}_{n}")
                nc.tensor.matmul(
                    pp[:], avT_tiles[b][:, jj * P:(jj + 1) * P],
                    wp_sb[:, n * 512:(n + 1) * 512],
                    start=True, stop=True)
                ostage = spool.tile([P, 512], f32, tag="ostage")
                nc.vector.tensor_copy(ostage[:], pp[:])
                jl = b * TTB + jj
                nc.sync.dma_start(
                    out[jl * P:(jl + 1) * P, n * 512:(n + 1) * 512],
                    ostage[:])

            def front_work(b):
                for mi in range(TTB):
                    yield lambda mi=mi: emit_qkv_m(b, mi)
                    if mi % 8 == 7:
                        g = mi // 8
                        yield lambda g=g: emit_rope_group(b, g)
                        for mj in range(g * 8, g * 8 + 8):
                            yield lambda mj=mj: emit_tr_m(b, mj)

            def tail_work(b):
                for jj in range(TTB):
                    yield lambda jj=jj: emit_avtr(b, jj)
                    yield lambda jj=jj: emit_proj(b, jj, 0)
                    yield lambda jj=jj: emit_proj(b, jj, 1)

            def pump(it, nmax):
                done = 0
                for th in it:
                    th()
                    done += 1
                    if done >= nmax:
                        return

            # ---- schedule --------------------------------------------
            from collections import deque
            bg_queue = deque()

            def pump(n):
                for _ in range(n):
                    if not bg_queue:
                        return
                    bg_queue.popleft()()

            for th in front_work(0):
                th()
            for th in front_work(1):
                bg_queue.append(th)

            blocks = [(b, h, qb) for b in range(B) for h in range(HPC)
                      for qb in range(4)]
            prev = None
            for bi, (b, h, qb) in enumerate(blocks):
                # ST groups with AV chains of the previous block and bg work
                # interleaved so PE never stalls on the exp ping-pong.
                prt = slice(h * D, (h + 1) * D)
                qc = b * N + qb * 512
                expst = wpool.tile([P, 16, 512], bf16, tag="expst",
                                   name=f"expst_{b}_{h}_{qb}")
                for g in range(8):
                    stp = ps.tile([P, 1024], f32, tag="st")
                    for s_ in range(2):
                        kc = g * 2 + s_
                        nc.tensor.matmul(
                            stp[:, s_ * 512:(s_ + 1) * 512],
                            kT_sb[prt, b * N + kc * P: b * N + (kc + 1) * P],
                            qT_sb[prt, qc:qc + 512],
                            start=True, stop=True)
                    nc.scalar.activation(
                        expst[:, g * 2:(g + 1) * 2, :], stp[:],
                        AF.Exp, scale=SCALE)
                    if g % 2 == 1:
                        if prev is not None:
                            emit_av_chain(*prev, qs=g // 2)
                        pump(1)
                if prev is not None:
                    pb, ph, pqb, _ = prev
                    if ph == 1:
                        for jj in range(pqb * 4, pqb * 4 + 4):
                            bg_queue.append(lambda pb=pb, jj=jj: emit_avtr(pb, jj))
                            bg_queue.append(lambda pb=pb, jj=jj: emit_proj(pb, jj, 0))
                            bg_queue.append(lambda pb=pb, jj=jj: emit_proj(pb, jj, 1))
                prev = (b, h, qb, expst)
            for qs in range(4):
                emit_av_chain(*prev, qs=qs)
            pb, ph, pqb, _ = prev
            for jj in range(pqb * 4, pqb * 4 + 4):
                bg_queue.append(lambda jj=jj: emit_avtr(1, jj))
                bg_queue.append(lambda jj=jj: emit_proj(1, jj, 0))
                bg_queue.append(lambda jj=jj: emit_proj(1, jj, 1))
            while bg_queue:
                bg_queue.popleft()()

    nc.finalize()
    return nc


_CACHED = {}


def _get_kernel():
    if "nc" not in _CACHED:
        _CACHED["nc"] = build_kernel()
    return _CACHED["nc"]


# ---------------------------------------------------------------------------
# host-side sharding / gather
# ---------------------------------------------------------------------------
def _deint(base, h):
    """qkv row indices for head h with rotary pairs deinterleaved."""
    ev = [base + D * h + 2 * i for i in range(32)]
    od = [base + D * h + 2 * i + 1 for i in range(32)]
    return ev + od


def make_in_maps(x, freqs_cos, freqs_sin, W_qkv, W_proj):
    x = np.asarray(x, dtype=np.float32)
    fc = np.asarray(freqs_cos, dtype=np.float32)
    fs = np.asarray(freqs_sin, dtype=np.float32)
    W_qkv = np.asarray(W_qkv, dtype=np.float32)
    W_proj = np.asarray(W_proj, dtype=np.float32)

    xf = x.reshape(T, DIM)
    # xt[m, k, p, c] = x[m*128+c, k*128+p]
    xt = np.ascontiguousarray(
        xf.reshape(TT, P, KC, P).transpose(0, 2, 3, 1)).astype(ml_dtypes.bfloat16)

    # rope tables: token tile j, partition p -> batch token (j % TTB)*128+p
    tokn = (np.arange(TT)[None, :] % TTB) * P + np.arange(P)[:, None]  # (P, TT)
    rc = np.ascontiguousarray(fc[tokn])          # (P, TT, 32)
    rs = np.ascontiguousarray(fs[tokn])
    ident = np.eye(P, dtype=np.float32).astype(ml_dtypes.bfloat16)

    in_maps = []
    for c in range(NCORE):
        h0, h1 = 2 * c, 2 * c + 1
        q_rows = _deint(0, h0) + _deint(0, h1)
        k_rows = _deint(DIM, h0) + _deint(DIM, h1)
        v_rows = ([2 * DIM + D * h0 + d for d in range(D)] +
                  [2 * DIM + D * h1 + d for d in range(D)])
        W_shard = W_qkv[q_rows + k_rows + v_rows, :]          # (384, 1024)
        # wq[p, k, f] = W_shard[f, k*128+p]
        wq = np.ascontiguousarray(
            W_shard.T.reshape(KC, P, QKVF).transpose(1, 0, 2)).astype(ml_dtypes.bfloat16)
        wp = np.ascontiguousarray(
            W_proj[:, P * c:P * (c + 1)].T).astype(ml_dtypes.bfloat16)
        in_maps.append(dict(xt=xt, wq=wq, wp=wp, rc=rc, rs=rs, ident=ident))
    return in_maps


def kernel(x, freqs_cos, freqs_sin, W_qkv, W_proj, b_proj, _trace=False):
    from concourse.bass_utils import run_bass_kernel_spmd

    nc = _get_kernel()
    in_maps = make_in_maps(x, freqs_cos, freqs_sin, W_qkv, W_proj)
    res = run_bass_kernel_spmd(nc, in_maps, list(range(NCORE)), trace=_trace)
    acc = np.zeros((T, DIM), dtype=np.float32)
    for c in range(NCORE):
        acc += res.results[c]["out"]
    acc += np.asarray(b_proj, dtype=np.float32)[None, :]
    outv = acc.reshape(B, N, DIM)
    if _trace:
        return outv, res
    return outv
